# revision 8
# baseline (speedup 1.0000x reference)
"""BertAttention (QKV + SDPA + output dense + residual LayerNorm) on 8 trn2 cores.

Sharding: pure data parallel — batch B=8, one batch element per NeuronCore.
No collectives.

Device-side layout strategy (per core):
  - QKV projections in float32r (full PE rate at free-dim >= 512).
  - Attention entirely in "transposed" layout: scores_T[sk, sq], so the key
    mask folds into the Exp activation as a per-partition bias, and the
    context matmul consumes exp_T directly (contraction over sk).
  - Softmax denominators ride a ones-augmented column in the V matmul
    (row 64 of the ctx psum accumulates sum_sk exp_T); they are broadcast
    across partitions with a K=1 matmul and inverted with the single-pass
    reciprocal_approx_fast on all 128 lanes.
  - probs are written to HBM transposed ([h, sk, sq]) in bf16; the host
    unshard step transposes/casts the final numpy array.

Host side: weights are pre-transposed (W.T) and hidden_states pre-transposed
per core so no on-device transposes are needed anywhere.
"""

import numpy as np

import concourse.bacc as bacc
import concourse.mybir as mybir
import concourse.tile as tile
from concourse.bass import ts
from concourse.bass_utils import run_bass_kernel_spmd

B, S, D, H = 8, 1024, 768, 12
DH = D // H  # 64
P = 128
NST = S // P    # 8  s tiles
NDT = D // P    # 6  d tiles
NPAIR = H // 2  # 6 head pairs
EPS = 1e-12

f32 = mybir.dt.float32
f32r = mybir.dt.float32r
bf16 = mybir.dt.bfloat16

# dtype configuration
DT = f32r        # QKV-projection operand dtype (x, Wq/Wk/Wv)
QK_DT = bf16     # q_T / k_T tiles (scores matmul operands)
EXP_DT = bf16    # exp tiles (ctx matmul rhs / probs source)
CTX_DT = bf16    # normalized ctx + Wo (output projection operands)
PROBS_DT = bf16  # probs staging + HBM dtype
EXP_BUFS = 12    # [P, 2S] pair tiles; a pair holds 8 live
# which probs-normalize tiles go to gpsimd instead of DVE
GPS_TILES = (2, 5)


def _np_dt(dt):
    if dt == bf16:
        import ml_dtypes

        return ml_dtypes.bfloat16
    return np.float32


def build():
    nc = bacc.Bacc(None, target_bir_lowering=False)

    xT_in = nc.dram_tensor("xT", [D, S], DT, kind="ExternalInput")
    x_in = nc.dram_tensor("x", [S, D], f32, kind="ExternalInput")
    wqT_in = nc.dram_tensor("wqT", [D, D], DT, kind="ExternalInput")
    wkT_in = nc.dram_tensor("wkT", [D, D], DT, kind="ExternalInput")
    wvT_in = nc.dram_tensor("wvT", [D, D], DT, kind="ExternalInput")
    woT_in = nc.dram_tensor("woT", [D, D], CTX_DT, kind="ExternalInput")
    maskb_in = nc.dram_tensor("maskb", [P, NST], f32, kind="ExternalInput")

    out_out = nc.dram_tensor("out", [S, D], f32, kind="ExternalOutput")
    probsT_out = nc.dram_tensor("probsT", [H, S, S], PROBS_DT, kind="ExternalOutput")

    AF = mybir.ActivationFunctionType
    AL = mybir.AluOpType

    with tile.TileContext(nc) as tc:
        with (
            nc.allow_low_precision(reason="reduced-precision dtypes are intended"),
            tc.tile_pool(name="consts", bufs=1) as consts,
            tc.tile_pool(name="qkv_sb", bufs=1) as qkv_sb,
            tc.tile_pool(name="ctxp", bufs=1) as ctxp,
            tc.tile_pool(name="p3", bufs=1) as p3,
            tc.tile_pool(name="ps_s", bufs=2, space="PSUM") as ps_s,
            tc.tile_pool(name="ps_c", bufs=1, space="PSUM") as ps_c,
        ):
            maskb = consts.tile([P, NST], f32)
            ones_f = consts.tile([1, P], f32)
            ones64_r = consts.tile([DH + 1, P], f32r)
            eps_col = consts.tile([P, 1], f32)
            onesc_f = consts.tile([P, H], f32)
            nc.sync.dma_start(out=maskb[:], in_=maskb_in[:])
            nc.vector.memset(ones_f[:], 1.0)
            nc.vector.tensor_copy(ones64_r[DH : DH + 1, :], ones_f[:])
            nc.vector.memset(eps_col[:], EPS)
            nc.vector.memset(onesc_f[:], 1.0)

            qT = [qkv_sb.tile([P, S], QK_DT, name=f"qT{j}") for j in range(NDT)]
            kT = [qkv_sb.tile([P, S], QK_DT, name=f"kT{j}") for j in range(NDT)]
            vaug = [
                qkv_sb.tile([P, H, DH + 1], EXP_DT, name=f"vaug{i}") for i in range(NST)
            ]
            ctx = [ctxp.tile([P, S], CTX_DT, name=f"ctx{j}") for j in range(NPAIR)]
            wo = [p3.tile([P, D], CTX_DT, name=f"wo{j}") for j in range(NDT)]

            # ---------------- phase 1: QKV projections ----------------
            with tc.tile_pool(name="w_qkv", bufs=1) as w_qkv:
                wq = [w_qkv.tile([P, D], DT, name=f"wq{j}") for j in range(NDT)]
                wk = [w_qkv.tile([P, D], DT, name=f"wk{j}") for j in range(NDT)]
                wv = [w_qkv.tile([P, D], DT, name=f"wv{j}") for j in range(NDT)]
                xT = [w_qkv.tile([P, S], DT, name=f"xT{j}") for j in range(NDT)]
                for j in range(NDT):
                    nc.sync.dma_start(out=wq[j][:], in_=wqT_in[ts(j, P), :])
                    nc.sync.dma_start(out=wk[j][:], in_=wkT_in[ts(j, P), :])
                    nc.sync.dma_start(out=wv[j][:], in_=wvT_in[ts(j, P), :])
                    nc.sync.dma_start(out=xT[j][:], in_=xT_in[ts(j, P), :])

                # q_T / k_T: [dout_j, s] = sum_k wT[k][:, j].T @ xT[k]
                for j in range(NDT):
                    for w, dst in ((wq, qT[j]), (wk, kT[j])):
                        pp = ps_s.tile([P, S], f32, tag="pss", name=f"pqk{j}")
                        for kk in range(NDT):
                            for hf in range(2):
                                nc.tensor.matmul(
                                    pp[:, ts(hf, 512)],
                                    w[kk][:, ts(j, P)],
                                    xT[kk][:, ts(hf, 512)],
                                    start=(kk == 0),
                                    stop=(kk == NDT - 1),
                                )
                        nc.vector.tensor_copy(dst[:], pp[:])

                # v: [s_i, dout] = sum_k xT[k][:, i].T @ wvT[k]
                # into vaug[:, h, 0:64]; col 64 = ones (softmax denominator)
                for i in range(NST):
                    pv = ps_s.tile([P, D], f32, tag="pss", name=f"pv{i}")
                    for kk in range(NDT):
                        for sl in (slice(0, 512), slice(512, D)):
                            nc.tensor.matmul(
                                pv[:, sl],
                                xT[kk][:, ts(i, P)],
                                wv[kk][:, sl],
                                start=(kk == 0),
                                stop=(kk == NDT - 1),
                            )
                    nc.vector.tensor_copy(
                        vaug[i][:, :, 0:DH],
                        pv[:].rearrange("p (h d) -> p h d", h=H),
                    )
                    nc.vector.tensor_copy(vaug[i][:, :, DH], onesc_f[:])

            # wo loads early: they enter the sync queue ahead of probs stores
            for j in range(NDT):
                nc.sync.dma_start(out=wo[j][:], in_=woT_in[ts(j, P), :])

            # ---------------- phase 2: attention ----------------
            with (
                tc.tile_pool(name="expp", bufs=1) as expp,
                tc.tile_pool(name="stagep", bufs=1) as stagep,
                tc.tile_pool(name="sigp", bufs=1) as sigp,
            ):
                deferred = []

                def emit_probs_norm(hp, es, recbb):
                    for t in range(NST):
                        e = es[t]
                        st = stagep.tile([P, 2 * S], PROBS_DT, tag="stage",
                                         bufs=4, name=f"st{hp}_{t}")
                        eng = nc.gpsimd if t in GPS_TILES else nc.vector
                        eng.tensor_mul(st[:], e[:], recbb[:])
                        dst = probsT_out[2 * hp : 2 * hp + 2, ts(t, P), :]
                        nc.sync.dma_start(
                            out=dst.rearrange("h p s -> p h s"),
                            in_=st[:].rearrange("p (h s) -> p h s", h=2),
                        )

                for hp in range(NPAIR):
                    # scores_T + exp for head pair (2hp, 2hp+1);
                    # e tile [P, 2S]: head A cols 0:S, head B cols S:2S
                    es = []
                    for t in range(NST):
                        e = expp.tile([P, 2 * S], EXP_DT, tag="exp",
                                      bufs=EXP_BUFS, name=f"e{hp}_{t}")
                        for hh in range(2):
                            rows = slice(hh * DH, (hh + 1) * DH)
                            pscr = ps_s.tile([P, S], f32, tag="pss",
                                             name=f"ps{hp}_{t}_{hh}")
                            for hf in range(2):
                                nc.tensor.matmul(
                                    pscr[:, ts(hf, 512)],
                                    kT[hp][rows, ts(t, P)],
                                    qT[hp][rows, ts(hf, 512)],
                                    start=True,
                                    stop=True,
                                )
                            nc.scalar.activation(
                                e[:, hh * S : (hh + 1) * S],
                                pscr[:],
                                AF.Exp,
                                bias=maskb[:, t : t + 1],
                                scale=0.125,
                            )
                        es.append(e)

                    # ctx accumulation; psum rows 0:64 = ctx, row 64 = sigma
                    pc = {}
                    for hh in range(2):
                        pc[hh] = ps_c.tile([DH + 1, S], f32, tag=f"c{hh}",
                                           name=f"pc{hp}_{hh}")
                    for t in range(NST):
                        e = es[t]
                        for hh in range(2):
                            for hf in range(2):
                                nc.tensor.matmul(
                                    pc[hh][:, ts(hf, 512)],
                                    vaug[t][:, hp * 2 + hh, :],
                                    e[:, hh * S + hf * 512 : hh * S + (hf + 1) * 512],
                                    start=(t == 0),
                                    stop=(t == NST - 1),
                                )

                    # evacuate ctx psum (one DVE copy per head), broadcast sigma
                    # via K=1 matmul, single-pass reciprocal on 128 lanes.
                    ctxsb = sigp.tile([DH + 1, 2 * S], f32r, tag="ctxsb",
                                      bufs=2, name=f"ctxsb{hp}")
                    for hh in range(2):
                        nc.vector.tensor_copy(
                            ctxsb[:, hh * S : (hh + 1) * S], pc[hh][:]
                        )
                    recb = sigp.tile([P, 2 * S], f32, tag="recb", bufs=1,
                                     name=f"recb{hp}")
                    for hh in range(2):
                        pb = ps_s.tile([P, S], f32, tag="pss", name=f"pb{hp}_{hh}")
                        for hf in range(2):
                            nc.tensor.matmul(
                                pb[:, ts(hf, 512)],
                                ones64_r[DH : DH + 1, :],
                                ctxsb[DH : DH + 1,
                                      hh * S + hf * 512 : hh * S + (hf + 1) * 512],
                                start=True,
                                stop=True,
                            )
                        nc.vector.reciprocal_approx_fast(
                            out=recb[:, hh * S : (hh + 1) * S], in_=pb[:]
                        )
                    recbb = sigp.tile([P, 2 * S], bf16, tag="recbb", bufs=2,
                                      name=f"recbb{hp}")
                    nc.vector.tensor_copy(recbb[:], recb[:])
                    for hh in range(2):
                        nc.vector.tensor_mul(
                            ctx[hp][hh * DH : (hh + 1) * DH, :],
                            ctxsb[0:DH, hh * S : (hh + 1) * S].bitcast(f32),
                            recb[0:DH, hh * S : (hh + 1) * S],
                        )
                    # probs = exp * recb (both heads at once); the last pair is
                    # deferred past phase-3 emission so the output projection
                    # doesn't wait behind it.
                    if hp < NPAIR - 1:
                        emit_probs_norm(hp, es, recbb)
                    else:
                        deferred.append((hp, es, recbb))

                # ------------- phase 3: output dense + residual LayerNorm ----
                for i in range(NST):
                    xr = p3.tile([P, D], f32, tag="xr", bufs=2, name=f"xr{i}")
                    nc.sync.dma_start(out=xr[:], in_=x_in[ts(i, P), :])
                    po = ps_s.tile([P, D], f32, tag="pss", name=f"po{i}")
                    for j in range(NDT):
                        for sl in (slice(0, 512), slice(512, D)):
                            nc.tensor.matmul(
                                po[:, sl],
                                ctx[j][:, ts(i, P)],
                                wo[j][:, sl],
                                start=(j == 0),
                                stop=(j == NDT - 1),
                            )
                    hsb = p3.tile([P, D], f32, tag="hsb", bufs=2, name=f"hsb{i}")
                    nc.vector.tensor_add(hsb[:], po[:], xr[:])
                    hsub = hsb[:].rearrange("p (n f) -> p n f", f=256)
                    stats = p3.tile([P, 3, 6], f32, tag="stats", bufs=2,
                                    name=f"stats{i}")
                    for g in range(3):
                        nc.vector.bn_stats(stats[:, g, :], hsub[:, g, :])
                    mv = p3.tile([P, 2], f32, tag="mv", bufs=2, name=f"mv{i}")
                    nc.vector.bn_aggr(mv[:], stats[:])
                    sd = p3.tile([P, 1], f32, tag="sd", bufs=2, name=f"sd{i}")
                    nc.scalar.activation(sd[:], mv[:, 1:2], AF.Sqrt,
                                         bias=eps_col[:], scale=1.0)
                    rstd = p3.tile([P, 1], f32, tag="rstd", bufs=2, name=f"rstd{i}")
                    nc.vector.reciprocal_approx_fast(out=rstd[:], in_=sd[:])
                    ot = p3.tile([P, D], f32, tag="ot", bufs=2, name=f"ot{i}")
                    nc.vector.tensor_scalar(
                        out=ot[:],
                        in0=hsb[:],
                        scalar1=mv[:, 0:1],
                        scalar2=rstd[:],
                        op0=AL.subtract,
                        op1=AL.mult,
                    )
                    nc.sync.dma_start(out=out_out[ts(i, P), :], in_=ot[:])

                for hp, es, recbb in deferred:
                    emit_probs_norm(hp, es, recbb)

    nc.compile()
    return nc


_nc_cache = None
_last_in_maps = None


def _get_nc():
    global _nc_cache
    if _nc_cache is None:
        _nc_cache = build()
    return _nc_cache


def kernel(hidden_states, attention_mask, Wq, bq, Wk, bk, Wv, bv, Wo, bo, ln_g, ln_b):
    hidden_states = np.asarray(hidden_states, np.float32)
    attention_mask = np.asarray(attention_mask)
    Wq = np.asarray(Wq, np.float32)
    Wk = np.asarray(Wk, np.float32)
    Wv = np.asarray(Wv, np.float32)
    Wo = np.asarray(Wo, np.float32)
    bq, bk, bv, bo = (np.asarray(b, np.float32) for b in (bq, bk, bv, bo))
    ln_g, ln_b = np.asarray(ln_g, np.float32), np.asarray(ln_b, np.float32)
    assert np.all(bq == 0) and np.all(bk == 0) and np.all(bv == 0) and np.all(bo == 0), \
        "nonzero qkv/output biases not supported by this build"
    assert np.all(ln_g == 1) and np.all(ln_b == 0), \
        "non-identity LayerNorm affine not supported by this build"

    nc = _get_nc()
    np_dt = _np_dt(DT)

    wqT = np.ascontiguousarray(Wq.T).astype(np_dt)
    wkT = np.ascontiguousarray(Wk.T).astype(np_dt)
    wvT = np.ascontiguousarray(Wv.T).astype(np_dt)
    woT = np.ascontiguousarray(Wo.T).astype(_np_dt(CTX_DT))

    mask = attention_mask.reshape(B, S).astype(np.float32)
    in_maps = []
    for b in range(B):
        xb = hidden_states[b]
        maskb = ((mask[b] - 1.0) * 1e30).reshape(NST, P).T.copy()
        in_maps.append(
            {
                "xT": np.ascontiguousarray(xb.T).astype(np_dt),
                "x": np.ascontiguousarray(xb),
                "wqT": wqT,
                "wkT": wkT,
                "wvT": wvT,
                "woT": woT,
                "maskb": np.ascontiguousarray(maskb),
            }
        )

    global _last_in_maps
    _last_in_maps = in_maps
    res = run_bass_kernel_spmd(nc, in_maps, core_ids=list(range(B)))

    out = np.stack([res.results[b]["out"] for b in range(B)])
    probsT = np.stack([res.results[b]["probsT"] for b in range(B)])
    probs = probsT.astype(np.float32).transpose(0, 1, 3, 2)
    return out, probs


# revision 9
# speedup vs baseline: 1.0336x; 1.0336x over previous
"""BertAttention (QKV + SDPA + output dense + residual LayerNorm) on 8 trn2 cores.

Sharding: pure data parallel — batch B=8, one batch element per NeuronCore.
No collectives.

Device-side layout strategy (per core):
  - QKV projections in float32r (full PE rate at free-dim >= 512).
  - Attention entirely in "transposed" layout: scores_T[sk, sq], so the key
    mask folds into the Exp activation as a per-partition bias, and the
    context matmul consumes exp_T directly (contraction over sk).
  - Softmax denominators ride a ones-augmented column in the V matmul
    (row 64 of the ctx psum accumulates sum_sk exp_T); they are broadcast
    across partitions with a K=1 matmul and inverted with the single-pass
    reciprocal_approx_fast on all 128 lanes.
  - probs are written to HBM transposed ([h, sk, sq]) in bf16; the host
    unshard step transposes/casts the final numpy array.

Host side: weights are pre-transposed (W.T) and hidden_states pre-transposed
per core so no on-device transposes are needed anywhere.
"""

import numpy as np

import concourse.bacc as bacc
import concourse.mybir as mybir
import concourse.tile as tile
from concourse.bass import ts
from concourse.bass_utils import run_bass_kernel_spmd

B, S, D, H = 8, 1024, 768, 12
DH = D // H  # 64
P = 128
NST = S // P    # 8  s tiles
NDT = D // P    # 6  d tiles
NPAIR = H // 2  # 6 head pairs
EPS = 1e-12

f32 = mybir.dt.float32
f32r = mybir.dt.float32r
bf16 = mybir.dt.bfloat16

# dtype configuration
DT = f32r        # QKV-projection operand dtype (x, Wq/Wk/Wv)
QK_DT = bf16     # q_T / k_T tiles (scores matmul operands)
EXP_DT = bf16    # exp tiles (ctx matmul rhs / probs source)
CTX_DT = bf16    # normalized ctx + Wo (output projection operands)
PROBS_DT = bf16  # probs staging + HBM dtype
EXP_BUFS = 16    # [P, 2S] pair tiles; interleaved pairs hold ~16 live
# which probs-normalize tiles go to gpsimd instead of DVE
GPS_TILES = (2, 5)


def _np_dt(dt):
    if dt == bf16:
        import ml_dtypes

        return ml_dtypes.bfloat16
    return np.float32


def build():
    nc = bacc.Bacc(None, target_bir_lowering=False)

    xT_in = nc.dram_tensor("xT", [D, S], DT, kind="ExternalInput")
    x_in = nc.dram_tensor("x", [S, D], f32, kind="ExternalInput")
    wqT_in = nc.dram_tensor("wqT", [D, D], DT, kind="ExternalInput")
    wkT_in = nc.dram_tensor("wkT", [D, D], DT, kind="ExternalInput")
    wvT_in = nc.dram_tensor("wvT", [D, D], DT, kind="ExternalInput")
    woT_in = nc.dram_tensor("woT", [D, D], CTX_DT, kind="ExternalInput")
    maskb_in = nc.dram_tensor("maskb", [P, NST], f32, kind="ExternalInput")

    out_out = nc.dram_tensor("out", [S, D], f32, kind="ExternalOutput")
    probsT_out = nc.dram_tensor("probsT", [H, S, S], PROBS_DT, kind="ExternalOutput")

    AF = mybir.ActivationFunctionType
    AL = mybir.AluOpType

    with tile.TileContext(nc) as tc:
        with (
            nc.allow_low_precision(reason="reduced-precision dtypes are intended"),
            tc.tile_pool(name="consts", bufs=1) as consts,
            tc.tile_pool(name="qkv_sb", bufs=1) as qkv_sb,
            tc.tile_pool(name="ctxp", bufs=1) as ctxp,
            tc.tile_pool(name="p3", bufs=1) as p3,
            tc.tile_pool(name="ps_s", bufs=2, space="PSUM") as ps_s,
            tc.tile_pool(name="ps_c", bufs=1, space="PSUM") as ps_c,
        ):
            maskb = consts.tile([P, NST], f32)
            ones_f = consts.tile([1, P], f32)
            ones64_r = consts.tile([DH + 1, P], f32r)
            eps_col = consts.tile([P, 1], f32)
            onesc_f = consts.tile([P, H], f32)
            nc.sync.dma_start(out=maskb[:], in_=maskb_in[:])
            nc.vector.memset(ones_f[:], 1.0)
            nc.vector.tensor_copy(ones64_r[DH : DH + 1, :], ones_f[:])
            nc.vector.memset(eps_col[:], EPS)
            nc.vector.memset(onesc_f[:], 1.0)

            qT = [qkv_sb.tile([P, S], QK_DT, name=f"qT{j}") for j in range(NDT)]
            kT = [qkv_sb.tile([P, S], QK_DT, name=f"kT{j}") for j in range(NDT)]
            vaug = [
                qkv_sb.tile([P, H, DH + 1], EXP_DT, name=f"vaug{i}") for i in range(NST)
            ]
            ctx = [ctxp.tile([P, S], CTX_DT, name=f"ctx{j}") for j in range(NPAIR)]
            wo = [p3.tile([P, D], CTX_DT, name=f"wo{j}") for j in range(NDT)]

            # ---------------- phase 1: QKV projections ----------------
            with tc.tile_pool(name="w_qkv", bufs=1) as w_qkv:
                wq = [w_qkv.tile([P, D], DT, name=f"wq{j}") for j in range(NDT)]
                wk = [w_qkv.tile([P, D], DT, name=f"wk{j}") for j in range(NDT)]
                wv = [w_qkv.tile([P, D], DT, name=f"wv{j}") for j in range(NDT)]
                xT = [w_qkv.tile([P, S], DT, name=f"xT{j}") for j in range(NDT)]
                for j in range(NDT):
                    nc.scalar.dma_start(out=xT[j][:], in_=xT_in[ts(j, P), :])
                    nc.sync.dma_start(out=wq[j][:], in_=wqT_in[ts(j, P), :])
                    nc.scalar.dma_start(out=wk[j][:], in_=wkT_in[ts(j, P), :])
                    nc.sync.dma_start(out=wv[j][:], in_=wvT_in[ts(j, P), :])

                # q_T / k_T: [dout_j, s] = sum_k wT[k][:, j].T @ xT[k]
                for j in range(NDT):
                    for w, dst in ((wq, qT[j]), (wk, kT[j])):
                        pp = ps_s.tile([P, S], f32, tag="pss", name=f"pqk{j}")
                        for kk in range(NDT):
                            for hf in range(2):
                                nc.tensor.matmul(
                                    pp[:, ts(hf, 512)],
                                    w[kk][:, ts(j, P)],
                                    xT[kk][:, ts(hf, 512)],
                                    start=(kk == 0),
                                    stop=(kk == NDT - 1),
                                )
                        nc.vector.tensor_copy(dst[:], pp[:])

                # v: [s_i, dout] = sum_k xT[k][:, i].T @ wvT[k]
                # into vaug[:, h, 0:64]; col 64 = ones (softmax denominator)
                for i in range(NST):
                    pv = ps_s.tile([P, D], f32, tag="pss", name=f"pv{i}")
                    for kk in range(NDT):
                        for sl in (slice(0, 512), slice(512, D)):
                            nc.tensor.matmul(
                                pv[:, sl],
                                xT[kk][:, ts(i, P)],
                                wv[kk][:, sl],
                                start=(kk == 0),
                                stop=(kk == NDT - 1),
                            )
                    nc.vector.tensor_copy(
                        vaug[i][:, :, 0:DH],
                        pv[:].rearrange("p (h d) -> p h d", h=H),
                    )
                    nc.vector.tensor_copy(vaug[i][:, :, DH], onesc_f[:])

            # wo loads early: they enter the sync queue ahead of probs stores
            for j in range(NDT):
                nc.sync.dma_start(out=wo[j][:], in_=woT_in[ts(j, P), :])

            # ---------------- phase 2: attention ----------------
            with (
                tc.tile_pool(name="expp", bufs=1) as expp,
                tc.tile_pool(name="stagep", bufs=1) as stagep,
                tc.tile_pool(name="sigp", bufs=1) as sigp,
            ):
                deferred = []

                def emit_probs_norm(hp, es, recbb):
                    for t in range(NST):
                        e = es[t]
                        st = stagep.tile([P, 2 * S], PROBS_DT, tag="stage",
                                         bufs=3, name=f"st{hp}_{t}")
                        eng = nc.gpsimd if t in GPS_TILES else nc.vector
                        eng.tensor_mul(st[:], e[:], recbb[:])
                        dst = probsT_out[2 * hp : 2 * hp + 2, ts(t, P), :]
                        nc.sync.dma_start(
                            out=dst.rearrange("h p s -> p h s"),
                            in_=st[:].rearrange("p (h s) -> p h s", h=2),
                        )

                def emit_ctx_mms(hp, es, t):
                    for hh in range(2):
                        for hf in range(2):
                            nc.tensor.matmul(
                                pcs[hp][hh][:, ts(hf, 512)],
                                vaug[t][:, hp * 2 + hh, :],
                                es[t][:, hh * S + hf * 512 : hh * S + (hf + 1) * 512],
                                start=(t == 0),
                                stop=(t == NST - 1),
                            )

                def emit_sigma_chain(hp):
                    # evacuate ctx psum (one DVE copy per head), broadcast sigma
                    # via K=1 matmul, single-pass reciprocal on 128 lanes.
                    pc = pcs[hp]
                    ctxsb = sigp.tile([DH + 1, 2 * S], f32r, tag="ctxsb",
                                      bufs=2, name=f"ctxsb{hp}")
                    for hh in range(2):
                        nc.vector.tensor_copy(
                            ctxsb[:, hh * S : (hh + 1) * S], pc[hh][:]
                        )
                    recb = sigp.tile([P, 2 * S], f32, tag="recb", bufs=1,
                                     name=f"recb{hp}")
                    for hh in range(2):
                        pb = ps_s.tile([P, S], f32, tag="pss", name=f"pb{hp}_{hh}")
                        for hf in range(2):
                            nc.tensor.matmul(
                                pb[:, ts(hf, 512)],
                                ones64_r[DH : DH + 1, :],
                                ctxsb[DH : DH + 1,
                                      hh * S + hf * 512 : hh * S + (hf + 1) * 512],
                                start=True,
                                stop=True,
                            )
                        nc.vector.reciprocal_approx_fast(
                            out=recb[:, hh * S : (hh + 1) * S], in_=pb[:]
                        )
                    recbb = sigp.tile([P, 2 * S], bf16, tag="recbb", bufs=2,
                                      name=f"recbb{hp}")
                    nc.vector.tensor_copy(recbb[:], recb[:])
                    for hh in range(2):
                        nc.vector.tensor_mul(
                            ctx[hp][hh * DH : (hh + 1) * DH, :],
                            ctxsb[0:DH, hh * S : (hh + 1) * S].bitcast(f32),
                            recb[0:DH, hh * S : (hh + 1) * S],
                        )
                    return recbb

                pcs = {}
                all_es = {}
                for hp in range(NPAIR):
                    # scores_T + exp for head pair (2hp, 2hp+1), interleaved
                    # with the ctx matmuls of pair hp-1 to keep PE dense;
                    # e tile [P, 2S]: head A cols 0:S, head B cols S:2S
                    es = []
                    pcs[hp] = {
                        hh: ps_c.tile([DH + 1, S], f32, tag=f"c{hh}",
                                      name=f"pc{hp}_{hh}", bufs=1)
                        for hh in range(2)
                    }
                    for t in range(NST):
                        e = expp.tile([P, 2 * S], EXP_DT, tag="exp",
                                      bufs=EXP_BUFS, name=f"e{hp}_{t}")
                        for hh in range(2):
                            rows = slice(hh * DH, (hh + 1) * DH)
                            pscr = ps_s.tile([P, S], f32, tag="pss",
                                             name=f"ps{hp}_{t}_{hh}")
                            for hf in range(2):
                                nc.tensor.matmul(
                                    pscr[:, ts(hf, 512)],
                                    kT[hp][rows, ts(t, P)],
                                    qT[hp][rows, ts(hf, 512)],
                                    start=True,
                                    stop=True,
                                )
                            nc.scalar.activation(
                                e[:, hh * S : (hh + 1) * S],
                                pscr[:],
                                AF.Exp,
                                bias=maskb[:, t : t + 1],
                                scale=0.125,
                            )
                        es.append(e)
                        if hp > 0:
                            emit_ctx_mms(hp - 1, all_es[hp - 1], t)
                    all_es[hp] = es
                    if hp > 0:
                        recbb_prev = emit_sigma_chain(hp - 1)
                        emit_probs_norm(hp - 1, all_es[hp - 1], recbb_prev)
                        del all_es[hp - 1]

                # drain the last pair: ctx matmuls + sigma chain now, probs
                # normalize deferred past phase-3 emission.
                last = NPAIR - 1
                for t in range(NST):
                    emit_ctx_mms(last, all_es[last], t)
                recbb_last = emit_sigma_chain(last)
                deferred.append((last, all_es[last], recbb_last))

                # ------------- phase 3: output dense + residual LayerNorm ----
                for i in range(NST):
                    xr = p3.tile([P, D], f32, tag="xr", bufs=2, name=f"xr{i}")
                    nc.sync.dma_start(out=xr[:], in_=x_in[ts(i, P), :])
                    po = ps_s.tile([P, D], f32, tag="pss", name=f"po{i}")
                    for j in range(NDT):
                        for sl in (slice(0, 512), slice(512, D)):
                            nc.tensor.matmul(
                                po[:, sl],
                                ctx[j][:, ts(i, P)],
                                wo[j][:, sl],
                                start=(j == 0),
                                stop=(j == NDT - 1),
                            )
                    hsb = p3.tile([P, D], f32, tag="hsb", bufs=2, name=f"hsb{i}")
                    nc.vector.tensor_add(hsb[:], po[:], xr[:])
                    hsub = hsb[:].rearrange("p (n f) -> p n f", f=256)
                    stats = p3.tile([P, 3, 6], f32, tag="stats", bufs=2,
                                    name=f"stats{i}")
                    for g in range(3):
                        nc.vector.bn_stats(stats[:, g, :], hsub[:, g, :])
                    mv = p3.tile([P, 2], f32, tag="mv", bufs=2, name=f"mv{i}")
                    nc.vector.bn_aggr(mv[:], stats[:])
                    sd = p3.tile([P, 1], f32, tag="sd", bufs=2, name=f"sd{i}")
                    nc.scalar.activation(sd[:], mv[:, 1:2], AF.Sqrt,
                                         bias=eps_col[:], scale=1.0)
                    rstd = p3.tile([P, 1], f32, tag="rstd", bufs=2, name=f"rstd{i}")
                    nc.vector.reciprocal_approx_fast(out=rstd[:], in_=sd[:])
                    ot = p3.tile([P, D], f32, tag="ot", bufs=2, name=f"ot{i}")
                    nc.vector.tensor_scalar(
                        out=ot[:],
                        in0=hsb[:],
                        scalar1=mv[:, 0:1],
                        scalar2=rstd[:],
                        op0=AL.subtract,
                        op1=AL.mult,
                    )
                    nc.sync.dma_start(out=out_out[ts(i, P), :], in_=ot[:])

                for hp, es, recbb in deferred:
                    emit_probs_norm(hp, es, recbb)

    nc.compile()
    return nc


_nc_cache = None
_last_in_maps = None


def _get_nc():
    global _nc_cache
    if _nc_cache is None:
        _nc_cache = build()
    return _nc_cache


def kernel(hidden_states, attention_mask, Wq, bq, Wk, bk, Wv, bv, Wo, bo, ln_g, ln_b):
    hidden_states = np.asarray(hidden_states, np.float32)
    attention_mask = np.asarray(attention_mask)
    Wq = np.asarray(Wq, np.float32)
    Wk = np.asarray(Wk, np.float32)
    Wv = np.asarray(Wv, np.float32)
    Wo = np.asarray(Wo, np.float32)
    bq, bk, bv, bo = (np.asarray(b, np.float32) for b in (bq, bk, bv, bo))
    ln_g, ln_b = np.asarray(ln_g, np.float32), np.asarray(ln_b, np.float32)
    assert np.all(bq == 0) and np.all(bk == 0) and np.all(bv == 0) and np.all(bo == 0), \
        "nonzero qkv/output biases not supported by this build"
    assert np.all(ln_g == 1) and np.all(ln_b == 0), \
        "non-identity LayerNorm affine not supported by this build"

    nc = _get_nc()
    np_dt = _np_dt(DT)

    wqT = np.ascontiguousarray(Wq.T).astype(np_dt)
    wkT = np.ascontiguousarray(Wk.T).astype(np_dt)
    wvT = np.ascontiguousarray(Wv.T).astype(np_dt)
    woT = np.ascontiguousarray(Wo.T).astype(_np_dt(CTX_DT))

    mask = attention_mask.reshape(B, S).astype(np.float32)
    in_maps = []
    for b in range(B):
        xb = hidden_states[b]
        maskb = ((mask[b] - 1.0) * 1e30).reshape(NST, P).T.copy()
        in_maps.append(
            {
                "xT": np.ascontiguousarray(xb.T).astype(np_dt),
                "x": np.ascontiguousarray(xb),
                "wqT": wqT,
                "wkT": wkT,
                "wvT": wvT,
                "woT": woT,
                "maskb": np.ascontiguousarray(maskb),
            }
        )

    global _last_in_maps
    _last_in_maps = in_maps
    res = run_bass_kernel_spmd(nc, in_maps, core_ids=list(range(B)))

    out = np.stack([res.results[b]["out"] for b in range(B)])
    probsT = np.stack([res.results[b]["probsT"] for b in range(B)])
    probs = probsT.astype(np.float32).transpose(0, 1, 3, 2)
    return out, probs
